# revision 5
# baseline (speedup 1.0000x reference)
"""Trainium2 Bass kernel for multi-head attention (B=4, N=2048, C=768, H=12).

Sharding: 8 cores = 4 batches x 2 sequence-halves. Each core computes K/V for
its batch's full 2048-token sequence (duplicated across the 2 cores sharing a
batch) and Q/attention/proj for its own 1024 query rows. No collectives; the
host gather is pure concatenation. The host passes x[b].T with the core's own
half rolled to the front, so Q-projection always reads columns 0:1024
(attention is permutation-invariant along keys, so rolling K/V is harmless).

v3: all-bf16 datapath (PSUM and the exp input stay fp32). bf16 stationary
operands get separate LDWEIGHTS, so the two 64-deep QK matmuls of a head pair
run concurrently as PE row tiles (0,0)/(64,0). V tiles are 65 columns (64 hd
+ ones row producing the softmax denominator in PSUM), so no memzero is
needed. ScalarE exp (25.2M elems/core at 1 elem/cyc/lane) is the pacing
engine; the kernel is structured so it starts early and never starves:
quad 0 runs attention chunks for its first head-pair immediately after each
token-block's projections (first ACT ~8us instead of ~59us), and quad 2's
attention interleaves with the final projection git-blocks. Softmax
normalization packs the 4 denominators of a (quad, ib) on partitions 0-3 and
does ONE DVE reciprocal (cost scales with free dim only), then GpSimd
partition_broadcast (sourced from partition 0 only) and a DVE multiply.
"""

import os
import ml_dtypes
import numpy as np

B, N, C = 4, 2048, 768
H, HD = 12, 64
SCALE = HD ** -0.5
P = 128
CT = C // P          # 6 contraction tiles
PAIRS = H // 2       # 6 head pairs
QUADS = H // 4       # 3 head quads
IQ = N // 2          # 1024 query rows per core
JT = N // P          # 16 key tiles
TKB = 512            # token-block width streamed from DRAM
VW = 72              # per-head stride in v_all (65 used: 64 hd + ones)
NCORES = 8

_cache = {}


def _build_bass():
    import concourse.bass as bass
    import concourse.tile as tile
    import concourse.mybir as mybir
    from concourse import bacc
    from concourse.bass import ts, ds
    from contextlib import ExitStack

    f32 = mybir.dt.float32
    bf16 = mybir.dt.bfloat16
    Exp = mybir.ActivationFunctionType.Exp

    nc = bacc.Bacc("TRN2", target_bir_lowering=False, debug=False)

    xt_d = nc.dram_tensor("xt", [C, N], bf16, kind="ExternalInput").ap()
    wq_d = nc.dram_tensor("wq", [C, C], bf16, kind="ExternalInput").ap()
    wk_d = nc.dram_tensor("wk", [C, C], bf16, kind="ExternalInput").ap()
    wv_d = nc.dram_tensor("wv", [C, C], bf16, kind="ExternalInput").ap()
    wp_d = nc.dram_tensor("wp", [C, C], bf16, kind="ExternalInput").ap()
    bb_d = nc.dram_tensor("bb", [P, C], f32, kind="ExternalInput").ap()
    out_d = nc.dram_tensor("out", [IQ, C], f32, kind="ExternalOutput").ap()

    xt_r = xt_d.rearrange("(o p) n -> p o n", p=P)
    wq_r = wq_d.rearrange("(o p) n -> p o n", p=P)
    wk_r = wk_d.rearrange("(o p) n -> p o n", p=P)
    wv_r = wv_d.rearrange("(o p) n -> p o n", p=P)
    wp_r = wp_d.rearrange("(o p) n -> p o n", p=P)
    out_r = out_d.rearrange("(t p) n -> t p n", p=P)

    with tile.TileContext(nc) as tc:
        with ExitStack() as ctx:
            persist = ctx.enter_context(tc.tile_pool(name="persist", bufs=1))
            outT_sb = persist.tile([P, PAIRS, IQ], bf16, name="outT_sb")
            v_all = persist.tile([P, JT, H * VW], bf16, name="v_all")
            v_all_r = v_all.rearrange("p t (h e) -> p t h e", e=VW)
            with nc.allow_low_precision(reason="ones column"):
                nc.vector.tensor_copy(
                    v_all_r[:, :, :, 64],
                    nc.const_aps.tensor(1.0, [P, JT, H], bf16),
                )

            wpool = ctx.enter_context(tc.tile_pool(name="wq", bufs=2))
            kvq = ctx.enter_context(tc.tile_pool(name="kvq", bufs=2))
            xt_pool = ctx.enter_context(tc.tile_pool(name="xtp", bufs=2))
            apsum = ctx.enter_context(
                tc.tile_pool(name="apsum", bufs=2, space="PSUM")
            )
            spsum = ctx.enter_context(
                tc.tile_pool(name="spsum", bufs=2, space="PSUM")
            )
            opsum = ctx.enter_context(
                tc.tile_pool(name="opsum", bufs=2, space="PSUM")
            )
            expt_pool = ctx.enter_context(tc.tile_pool(name="expt", bufs=4))
            nrm_pool = ctx.enter_context(tc.tile_pool(name="nrm", bufs=2))
            poS_pool = ctx.enter_context(tc.tile_pool(name="poSp", bufs=4))
            ppool = ctx.enter_context(tc.tile_pool(name="pw", bufs=1))
            outsb_pool = ctx.enter_context(tc.tile_pool(name="outsb", bufs=2))

            wp_sb = None
            bias_sb = None

            def attn_jts(kT_q, qT_q, q, tl, ib, pos, jts):
                t = 2 * q + tl
                for jt in jts:
                    ss = spsum.tile([P, 1024], f32, tag="ss", name="ss")
                    nc.tensor.matmul(
                        ss[:, 0:512],
                        kT_q[0:64, tl, ts(jt, P)],
                        qT_q[0:64, tl, ts(ib, 512)],
                        start=True,
                        stop=True,
                    )
                    nc.tensor.matmul(
                        ss[:, 512:1024],
                        kT_q[64:128, tl, ts(jt, P)],
                        qT_q[64:128, tl, ts(ib, 512)],
                        start=True,
                        stop=True,
                    )
                    et = expt_pool.tile([P, 1024], bf16, tag="et", name="et")
                    nc.scalar.activation(et[:], ss[:], Exp, scale=SCALE)
                    for hh in range(2):
                        hg = 2 * t + hh
                        nc.tensor.matmul(
                            pos[hh][0:65, :],
                            v_all_r[:, jt, hg, 0:65],
                            et[:, hh * 512 : (hh + 1) * 512],
                            start=(jt == 0),
                            stop=(jt == JT - 1),
                        )

            def finish_block(tl, pos, dpk, poS_acc):
                for hh in range(2):
                    poS = poS_pool.tile([65, 512], f32, tag="poS", name="poS")
                    nc.vector.tensor_copy(poS[:], pos[hh][0:65, :])
                    idx = 2 * tl + hh
                    nc.sync.dma_start(dpk[idx : idx + 1, :], poS[64:65, :])
                    poS_acc.append(poS)

            def normalize(q, ib, dpk, poS_acc):
                rd_q = nrm_pool.tile([4, 512], f32, tag="rd_q", name="rd_q")
                nc.vector.reciprocal(rd_q[:], dpk[:])
                for tl in range(2):
                    t = 2 * q + tl
                    for hh in range(2):
                        idx = 2 * tl + hh
                        poS = poS_acc[idx]
                        if idx == 0:
                            rd_src = rd_q
                        else:
                            # relocate to partition 0: HW partition_broadcast
                            # only sources partition 0 correctly
                            rd_src = nrm_pool.tile(
                                [1, 512], f32, tag="rd1", name="rd1"
                            )
                            nc.sync.dma_start(rd_src[:], rd_q[idx : idx + 1, :])
                        rb_sb = nrm_pool.tile([64, 512], f32, tag="rb_sb", name="rb_sb")
                        nc.gpsimd.partition_broadcast(rb_sb[:], rd_src[0:1, :])
                        with nc.allow_low_precision(reason="bf16 out path"):
                            nc.vector.tensor_mul(
                                outT_sb[hh * 64 : (hh + 1) * 64, t, ts(ib, 512)],
                                poS[0:64, :],
                                rb_sb[:],
                            )

            def final_git(git):
                ob = outsb_pool.tile([P, C], f32, tag="ob", name="ob")
                for n0, n1 in ((0, 512), (512, 768)):
                    pp = apsum.tile([P, 512], f32, tag="aps", name="pp")
                    for t in range(PAIRS):
                        nc.tensor.matmul(
                            pp[:, 0 : n1 - n0],
                            outT_sb[:, t, ds(git * P, P)],
                            wp_sb[:, t, n0:n1],
                            start=(t == 0),
                            stop=(t == PAIRS - 1),
                        )
                    nc.vector.tensor_add(
                        ob[:, n0:n1], pp[:, 0 : n1 - n0], bias_sb[:, n0:n1]
                    )
                nc.sync.dma_start(out_r[git], ob[:])

            for q in range(QUADS):
                # ---- load this quad's weight slices ----
                wk_t = wpool.tile([P, CT, 256], bf16, tag="wk_t")
                nc.sync.dma_start(wk_t[:], wk_r[:, :, ts(q, 256)])
                wq_t = wpool.tile([P, CT, 256], bf16, tag="wq_t")
                nc.sync.dma_start(wq_t[:], wq_r[:, :, ts(q, 256)])
                if q == 0:
                    wv_t = wpool.tile([P, CT, 512], bf16, tag="wv_t", name="wv_t")
                    nc.sync.dma_start(wv_t[:], wv_r[:, :, 0:512])
                elif q == 1:
                    wv_t = wpool.tile([P, CT, 256], bf16, tag="wv_t", name="wv_t")
                    nc.sync.dma_start(wv_t[:], wv_r[:, :, 512:768])
                if q == 2:
                    # stage final-projection weights during quad 2
                    wp_sb = ppool.tile([P, CT, C], bf16, name="wp_sb")
                    nc.sync.dma_start(wp_sb[:], wp_r)
                    bias_sb = ppool.tile([P, C], f32, name="bias_sb")
                    nc.sync.dma_start(bias_sb[:], bb_d)

                kT_q = kvq.tile([P, 2, N], bf16, tag="kT_q")
                qT_q = kvq.tile([P, 2, IQ], bf16, tag="qT_q")

                if q == 0:
                    early_pos = (
                        opsum.tile([P, 512], f32, tag="po", name="po0"),
                        opsum.tile([P, 512], f32, tag="po", name="po1"),
                    )

                # ---- projections (streamed over token blocks) ----
                for tb in range(N // TKB):
                    xt_t = xt_pool.tile([P, CT, TKB], bf16, tag="xt")
                    nc.sync.dma_start(xt_t[:], xt_r[:, :, ts(tb, TKB)])
                    for tl in range(2):
                        ps = apsum.tile([P, TKB], f32, tag="aps")
                        for c in range(CT):
                            nc.tensor.matmul(
                                ps[:],
                                wk_t[:, c, ts(tl, P)],
                                xt_t[:, c, :],
                                start=(c == 0),
                                stop=(c == CT - 1),
                            )
                        with nc.allow_low_precision(reason="bf16 k path"):
                            nc.vector.tensor_copy(kT_q[:, tl, ts(tb, TKB)], ps[:])
                    if tb < IQ // TKB:
                        for tl in range(2):
                            ps = apsum.tile([P, TKB], f32, tag="aps")
                            for c in range(CT):
                                nc.tensor.matmul(
                                    ps[:],
                                    wq_t[:, c, ts(tl, P)],
                                    xt_t[:, c, :],
                                    start=(c == 0),
                                    stop=(c == CT - 1),
                                )
                            with nc.allow_low_precision(reason="bf16 q path"):
                                nc.vector.tensor_copy(
                                    qT_q[:, tl, ts(tb, TKB)], ps[:]
                                )
                    if q < 2:
                        vn = 512 if q == 0 else 256
                        h0 = 0 if q == 0 else 8
                        for tt in range(TKB // P):
                            ps = apsum.tile([P, vn], f32, tag="aps")
                            for c in range(CT):
                                nc.tensor.matmul(
                                    ps[:],
                                    xt_t[:, c, ts(tt, P)],
                                    wv_t[:, c, 0:vn],
                                    start=(c == 0),
                                    stop=(c == CT - 1),
                                )
                            gtt = (tb * TKB) // P + tt
                            with nc.allow_low_precision(reason="bf16 v path"):
                                nc.vector.tensor_copy(
                                    v_all_r[:, gtt, h0 : h0 + vn // 64, 0:64],
                                    ps.rearrange("p (h e) -> p h e", e=64),
                                )
                    if q == 0:
                        # early attention on (ib0, tl0) over this tb's keys:
                        # gets ScalarE going ~50us earlier
                        attn_jts(
                            kT_q, qT_q, 0, 0, 0, early_pos,
                            range(4 * tb, 4 * tb + 4),
                        )

                # ---- attention blocks (ib-major), normalize per ib ----
                for ib in range(2):
                    poS_acc = []
                    dpk = nrm_pool.tile([4, 512], f32, tag="dpk", name="dpk")
                    for tl in range(2):
                        if q == 0 and ib == 0 and tl == 0:
                            pos = early_pos  # jts already emitted above
                        else:
                            pos = (
                                opsum.tile([P, 512], f32, tag="po", name="po0"),
                                opsum.tile([P, 512], f32, tag="po", name="po1"),
                            )
                            attn_jts(kT_q, qT_q, q, tl, ib, pos, range(JT))
                        finish_block(tl, pos, dpk, poS_acc)
                    normalize(q, ib, dpk, poS_acc)
                    if q == 2:
                        # final projection for this ib's git blocks
                        for git in range(4 * ib, 4 * ib + 4):
                            final_git(git)

    nc.compile()
    return nc


def _get_nc():
    if "nc" not in _cache:
        _cache["nc"] = _build_bass()
    return _cache["nc"]


def _prep_in_maps(x, w_qkv, w_proj, b_proj):
    x = np.asarray(x, np.float32)
    w_qkv = np.asarray(w_qkv, np.float32)
    w_proj = np.asarray(w_proj, np.float32)
    b_proj = np.asarray(b_proj, np.float32)

    bf = ml_dtypes.bfloat16
    wq = np.ascontiguousarray(w_qkv[0:C].T).astype(bf)
    wk = np.ascontiguousarray(w_qkv[C : 2 * C].T).astype(bf)
    wv = np.ascontiguousarray(w_qkv[2 * C : 3 * C].T).astype(bf)
    wp = np.ascontiguousarray(w_proj.T).astype(bf)
    bb = np.ascontiguousarray(np.broadcast_to(b_proj[None, :], (P, C)))

    in_maps = []
    for core in range(NCORES):
        b, half = core // 2, core % 2
        xT = x[b].T  # [C, N]
        mine = xT[:, half * IQ : (half + 1) * IQ]
        other = xT[:, (1 - half) * IQ : (2 - half) * IQ]
        xt = np.ascontiguousarray(np.concatenate([mine, other], axis=1)).astype(bf)
        in_maps.append(
            {"xt": xt, "wq": wq, "wk": wk, "wv": wv, "wp": wp, "bb": bb}
        )
    return in_maps


def run(x, w_qkv, w_proj, b_proj, trace=False):
    from concourse import bass_utils

    nc = _get_nc()
    in_maps = _prep_in_maps(x, w_qkv, w_proj, b_proj)
    br = bass_utils.run_bass_kernel_spmd(
        nc, in_maps, core_ids=list(range(NCORES)), trace=trace
    )
    y = np.empty((B, N, C), np.float32)
    for core in range(NCORES):
        b, half = core // 2, core % 2
        y[b, half * IQ : (half + 1) * IQ, :] = br.results[core]["out"]
    return y, br


def kernel(x, w_qkv, w_proj, b_proj):
    y, _ = run(x, w_qkv, w_proj, b_proj, trace=False)
    return y


# revision 8
# speedup vs baseline: 1.0085x; 1.0085x over previous
"""Trainium2 Bass kernel for multi-head attention (B=4, N=2048, C=768, H=12).

Sharding: 8 cores = 4 batches x 2 sequence-halves. Each core computes K/V for
its batch's full 2048-token sequence (duplicated across the 2 cores sharing a
batch) and Q/attention/proj for its own 1024 query rows. No collectives; the
host gather is pure concatenation. The host passes x[b].T with the core's own
half rolled to the front, so Q-projection always reads columns 0:1024
(attention is permutation-invariant along keys, so rolling K/V is harmless).

v4: all-bf16 datapath (PSUM and the exp input stay fp32). bf16 stationary
operands get separate LDWEIGHTS, so the two 64-deep QK matmuls of a head pair
run concurrently as PE row tiles (0,0)/(64,0). V tiles are 65 columns (64 hd
+ ones row producing the softmax denominator in PSUM), so no memzero is
needed. ScalarE exp (25.2M elems/core at 1 elem/cyc/lane, ~213us) is the
pacing engine; structure keeps it saturated: startup DMAs split across the
Sync and Activation HWDGE queues, quad 0 starts attention on its first head
pair right after (K, Q-tl0) of each token block, quad 2's attention
interleaves with final-projection pre-accumulation (head pairs 0-4), and
only the last pair's matmul + bias lands after the last softmax
normalization. Normalization is per head-pair: denominators packed on
partitions 0-1 by tiny DMAs, one DVE reciprocal (cost scales with free dim),
GpSimd partition_broadcast (sourced from partition 0 only), DVE multiply.
"""

import os
import ml_dtypes
import numpy as np

B, N, C = 4, 2048, 768
H, HD = 12, 64
SCALE = HD ** -0.5
P = 128
CT = C // P          # 6 contraction tiles
PAIRS = H // 2       # 6 head pairs
QUADS = H // 4       # 3 head quads
IQ = N // 2          # 1024 query rows per core
JT = N // P          # 16 key tiles
TKB = 512            # token-block width streamed from DRAM
VW = 72              # per-head stride in v_all (65 used: 64 hd + ones)
NCORES = 8

_cache = {}


def _build_bass():
    import concourse.bass as bass
    import concourse.tile as tile
    import concourse.mybir as mybir
    from concourse import bacc
    from concourse.bass import ts, ds
    from contextlib import ExitStack

    f32 = mybir.dt.float32
    bf16 = mybir.dt.bfloat16
    Exp = mybir.ActivationFunctionType.Exp

    nc = bacc.Bacc("TRN2", target_bir_lowering=False, debug=False)

    xt_d = nc.dram_tensor("xt", [C, N], bf16, kind="ExternalInput").ap()
    wq_d = nc.dram_tensor("wq", [C, C], bf16, kind="ExternalInput").ap()
    wk_d = nc.dram_tensor("wk", [C, C], bf16, kind="ExternalInput").ap()
    wv_d = nc.dram_tensor("wv", [C, C], bf16, kind="ExternalInput").ap()
    wp_d = nc.dram_tensor("wp", [C, C], bf16, kind="ExternalInput").ap()
    bb_d = nc.dram_tensor("bb", [P, C], f32, kind="ExternalInput").ap()
    out_d = nc.dram_tensor("out", [IQ, C], f32, kind="ExternalOutput").ap()

    xt_r = xt_d.rearrange("(o p) n -> p o n", p=P)
    wq_r = wq_d.rearrange("(o p) n -> p o n", p=P)
    wk_r = wk_d.rearrange("(o p) n -> p o n", p=P)
    wv_r = wv_d.rearrange("(o p) n -> p o n", p=P)
    wp_r = wp_d.rearrange("(o p) n -> p o n", p=P)
    out_r = out_d.rearrange("(t p) n -> t p n", p=P)

    with tile.TileContext(nc) as tc:
        with ExitStack() as ctx:
            persist = ctx.enter_context(tc.tile_pool(name="persist", bufs=1))
            outT_sb = persist.tile([P, PAIRS, IQ], bf16, name="outT_sb")
            v_all = persist.tile([P, JT, H * VW], bf16, name="v_all")
            v_all_r = v_all.rearrange("p t (h e) -> p t h e", e=VW)
            with nc.allow_low_precision(reason="ones column"):
                nc.vector.tensor_copy(
                    v_all_r[:, :, :, 64],
                    nc.const_aps.tensor(1.0, [P, JT, H], bf16),
                )

            wpool = ctx.enter_context(tc.tile_pool(name="wq", bufs=2))
            kvq = ctx.enter_context(tc.tile_pool(name="kvq", bufs=2))
            xt_pool = ctx.enter_context(tc.tile_pool(name="xtp", bufs=2))
            apsum = ctx.enter_context(
                tc.tile_pool(name="apsum", bufs=2, space="PSUM")
            )
            spsum = ctx.enter_context(
                tc.tile_pool(name="spsum", bufs=2, space="PSUM")
            )
            opsum = ctx.enter_context(
                tc.tile_pool(name="opsum", bufs=2, space="PSUM")
            )
            expt_pool = ctx.enter_context(tc.tile_pool(name="expt", bufs=4))
            nrm_pool = ctx.enter_context(tc.tile_pool(name="nrm", bufs=2))
            poS_pool = ctx.enter_context(tc.tile_pool(name="poSp", bufs=4))
            ppool = ctx.enter_context(tc.tile_pool(name="pw", bufs=1))
            outsb_pool = ctx.enter_context(tc.tile_pool(name="outsb", bufs=2))

            wp_sb = None
            bias_sb = None

            def attn_qk(kT_q, qT_q, tl, ib, jts):
                ets = []
                for jt in jts:
                    ss = spsum.tile([P, 1024], f32, tag="ss", name="ss")
                    nc.tensor.matmul(
                        ss[:, 0:512],
                        kT_q[0:64, tl, ts(jt, P)],
                        qT_q[0:64, tl, ts(ib, 512)],
                        start=True,
                        stop=True,
                    )
                    nc.tensor.matmul(
                        ss[:, 512:1024],
                        kT_q[64:128, tl, ts(jt, P)],
                        qT_q[64:128, tl, ts(ib, 512)],
                        start=True,
                        stop=True,
                    )
                    et = expt_pool.tile([P, 1024], bf16, tag="et", name="et")
                    nc.scalar.activation(et[:], ss[:], Exp, scale=SCALE)
                    ets.append((jt, et))
                return ets

            def attn_av(q, tl, pos, ets):
                t = 2 * q + tl
                for jt, et in ets:
                    for hh in range(2):
                        hg = 2 * t + hh
                        nc.tensor.matmul(
                            pos[hh][0:65, :],
                            v_all_r[:, jt, hg, 0:65],
                            et[:, hh * 512 : (hh + 1) * 512],
                            start=(jt == 0),
                            stop=(jt == JT - 1),
                        )

            def attn_jts(kT_q, qT_q, q, tl, ib, pos, jts):
                attn_av(q, tl, pos, attn_qk(kT_q, qT_q, tl, ib, jts))

            def norm_tl(q, tl, ib, pos):
                """Per-pair softmax normalization: outT = po[0:64] / po[64]."""
                t = 2 * q + tl
                dpk = nrm_pool.tile([2, 512], f32, tag="dpk", name="dpk")
                poSs = []
                for hh in range(2):
                    poS = poS_pool.tile([65, 512], f32, tag="poS", name="poS")
                    nc.vector.tensor_copy(poS[:], pos[hh][0:65, :])
                    nc.sync.dma_start(dpk[hh : hh + 1, :], poS[64:65, :])
                    poSs.append(poS)
                rd_q = nrm_pool.tile([2, 512], f32, tag="rd_q", name="rd_q")
                nc.vector.reciprocal(rd_q[:], dpk[:])
                for hh in range(2):
                    if hh == 0:
                        rd_src = rd_q
                    else:
                        # relocate to partition 0: HW partition_broadcast
                        # only sources partition 0 correctly
                        rd_src = nrm_pool.tile([1, 512], f32, tag="rd1", name="rd1")
                        nc.sync.dma_start(rd_src[:], rd_q[1:2, :])
                    rb_sb = nrm_pool.tile([64, 512], f32, tag="rb_sb", name="rb_sb")
                    nc.gpsimd.partition_broadcast(rb_sb[:], rd_src[0:1, :])
                    with nc.allow_low_precision(reason="bf16 out path"):
                        nc.vector.tensor_mul(
                            outT_sb[hh * 64 : (hh + 1) * 64, t, ts(ib, 512)],
                            poSs[hh][0:64, :],
                            rb_sb[:],
                        )

            def final_git_pre(git):
                """Accumulate head pairs 0..4 of the output projection."""
                pps = []
                for n0, n1 in ((0, 512), (512, 768)):
                    pp = apsum.tile([P, 512], f32, tag="aps", name="pp")
                    for t in range(PAIRS - 1):
                        nc.tensor.matmul(
                            pp[:, 0 : n1 - n0],
                            outT_sb[:, t, ds(git * P, P)],
                            wp_sb[:, t, n0:n1],
                            start=(t == 0),
                            stop=False,
                        )
                    pps.append(pp)
                return pps

            def final_git_post(git, pps, tail=False):
                """Last head pair + bias. The out DMA rides the Activation
                HWDGE queue only in the tail (after the last exp) — earlier it
                would block the ACT instruction stream."""
                ob = outsb_pool.tile([P, C], f32, tag="ob", name="ob")
                for (n0, n1), pp in zip(((0, 512), (512, 768)), pps):
                    nc.tensor.matmul(
                        pp[:, 0 : n1 - n0],
                        outT_sb[:, PAIRS - 1, ds(git * P, P)],
                        wp_sb[:, PAIRS - 1, n0:n1],
                        start=False,
                        stop=True,
                    )
                    nc.vector.tensor_add(
                        ob[:, n0:n1], pp[:, 0 : n1 - n0], bias_sb[:, n0:n1]
                    )
                (nc.scalar if tail else nc.sync).dma_start(out_r[git], ob[:])

            for q in range(QUADS):
                # ---- load this quad's weight slices (two HWDGE queues) ----
                wk_t = wpool.tile([P, CT, 256], bf16, tag="wk_t")
                wq_t = wpool.tile([P, CT, 256], bf16, tag="wq_t")
                if q == 0:
                    # parallelize the cold-start loads: wk/wq on the
                    # Activation queue, xt/wv on Sync
                    nc.scalar.dma_start(wk_t[:], wk_r[:, :, ts(q, 256)])
                    nc.scalar.dma_start(wq_t[:], wq_r[:, :, ts(q, 256)])
                else:
                    nc.sync.dma_start(wk_t[:], wk_r[:, :, ts(q, 256)])
                    nc.sync.dma_start(wq_t[:], wq_r[:, :, ts(q, 256)])
                if q == 0:
                    wv_t = wpool.tile([P, CT, 512], bf16, tag="wv_t", name="wv_t")
                    nc.sync.dma_start(wv_t[:], wv_r[:, :, 0:512])
                elif q == 1:
                    wv_t = wpool.tile([P, CT, 256], bf16, tag="wv_t", name="wv_t")
                    nc.sync.dma_start(wv_t[:], wv_r[:, :, 512:768])
                if q == 2:
                    # stage final-projection weights during quad 2
                    wp_sb = ppool.tile([P, CT, C], bf16, name="wp_sb")
                    nc.sync.dma_start(wp_sb[:], wp_r)
                    bias_sb = ppool.tile([P, C], f32, name="bias_sb")
                    nc.sync.dma_start(bias_sb[:], bb_d)

                kT_q = kvq.tile([P, 2, N], bf16, tag="kT_q")
                qT_q = kvq.tile([P, 2, IQ], bf16, tag="qT_q")

                if q == 0:
                    early_pos = (
                        opsum.tile([P, 512], f32, tag="po", name="po0"),
                        opsum.tile([P, 512], f32, tag="po", name="po1"),
                    )

                # ---- projections (streamed over token blocks) ----
                for tb in range(N // TKB):
                    xt_t = xt_pool.tile([P, CT, TKB], bf16, tag="xt")
                    nc.sync.dma_start(xt_t[:], xt_r[:, :, ts(tb, TKB)])

                    def k_group(tl):
                        ps = apsum.tile([P, TKB], f32, tag="aps", name="ps")
                        for c in range(CT):
                            nc.tensor.matmul(
                                ps[:],
                                wk_t[:, c, ts(tl, P)],
                                xt_t[:, c, :],
                                start=(c == 0),
                                stop=(c == CT - 1),
                            )
                        with nc.allow_low_precision(reason="bf16 k path"):
                            nc.vector.tensor_copy(kT_q[:, tl, ts(tb, TKB)], ps[:])

                    def q_group(tl):
                        ps = apsum.tile([P, TKB], f32, tag="aps", name="ps")
                        for c in range(CT):
                            nc.tensor.matmul(
                                ps[:],
                                wq_t[:, c, ts(tl, P)],
                                xt_t[:, c, :],
                                start=(c == 0),
                                stop=(c == CT - 1),
                            )
                        with nc.allow_low_precision(reason="bf16 q path"):
                            nc.vector.tensor_copy(qT_q[:, tl, ts(tb, TKB)], ps[:])

                    def v_groups():
                        vn = 512 if q == 0 else 256
                        h0 = 0 if q == 0 else 8
                        for tt in range(TKB // P):
                            ps = apsum.tile([P, vn], f32, tag="aps", name="ps")
                            for c in range(CT):
                                nc.tensor.matmul(
                                    ps[:],
                                    xt_t[:, c, ts(tt, P)],
                                    wv_t[:, c, 0:vn],
                                    start=(c == 0),
                                    stop=(c == CT - 1),
                                )
                            gtt = (tb * TKB) // P + tt
                            with nc.allow_low_precision(reason="bf16 v path"):
                                nc.vector.tensor_copy(
                                    v_all_r[:, gtt, h0 : h0 + vn // 64, 0:64],
                                    ps.rearrange("p (h e) -> p h e", e=64),
                                )

                    k_group(0)
                    k_group(1)
                    if tb < IQ // TKB:
                        q_group(0)
                    if q == 0:
                        # early attention on (ib0, tl0) over this tb's keys:
                        # QK+exp need only K + Q-tl0 of this tb — gets
                        # ScalarE going right after the first token block.
                        # The AV matmuls are emitted after the V groups
                        # (program order defines RAW semantics on v_all).
                        early_ets = attn_qk(
                            kT_q, qT_q, 0, 0, range(4 * tb, 4 * tb + 4)
                        )
                    if tb < IQ // TKB:
                        q_group(1)
                    if q < 2:
                        v_groups()
                    if q == 0:
                        attn_av(0, 0, early_pos, early_ets)

                # ---- attention blocks; per-pair normalize ----
                for ib in range(2):
                    pre_acc = []
                    for tl in range(2):
                        if q == 0 and ib == 0 and tl == 0:
                            pos = early_pos  # jts already emitted above
                        else:
                            pos = (
                                opsum.tile([P, 512], f32, tag="po", name="po0"),
                                opsum.tile([P, 512], f32, tag="po", name="po1"),
                            )
                            attn_jts(kT_q, qT_q, q, tl, ib, pos, range(JT))
                        norm_tl(q, tl, ib, pos)
                        if q == 2 and tl == 0:
                            # pre-accumulate two output blocks while the
                            # second pair's attention runs
                            pre_acc = [
                                (4 * ib + 0, final_git_pre(4 * ib + 0)),
                                (4 * ib + 1, final_git_pre(4 * ib + 1)),
                            ]
                    if q == 2:
                        tail = ib == 1
                        for git, pps in pre_acc:
                            final_git_post(git, pps, tail=tail)
                        for git in (4 * ib + 2, 4 * ib + 3):
                            final_git_post(git, final_git_pre(git), tail=tail)

    nc.compile()
    return nc


def _get_nc():
    if "nc" not in _cache:
        _cache["nc"] = _build_bass()
    return _cache["nc"]


def _prep_in_maps(x, w_qkv, w_proj, b_proj):
    x = np.asarray(x, np.float32)
    w_qkv = np.asarray(w_qkv, np.float32)
    w_proj = np.asarray(w_proj, np.float32)
    b_proj = np.asarray(b_proj, np.float32)

    bf = ml_dtypes.bfloat16
    wq = np.ascontiguousarray(w_qkv[0:C].T).astype(bf)
    wk = np.ascontiguousarray(w_qkv[C : 2 * C].T).astype(bf)
    wv = np.ascontiguousarray(w_qkv[2 * C : 3 * C].T).astype(bf)
    wp = np.ascontiguousarray(w_proj.T).astype(bf)
    bb = np.ascontiguousarray(np.broadcast_to(b_proj[None, :], (P, C)))

    in_maps = []
    for core in range(NCORES):
        b, half = core // 2, core % 2
        xT = x[b].T  # [C, N]
        mine = xT[:, half * IQ : (half + 1) * IQ]
        other = xT[:, (1 - half) * IQ : (2 - half) * IQ]
        xt = np.ascontiguousarray(np.concatenate([mine, other], axis=1)).astype(bf)
        in_maps.append(
            {"xt": xt, "wq": wq, "wk": wk, "wv": wv, "wp": wp, "bb": bb}
        )
    return in_maps


def run(x, w_qkv, w_proj, b_proj, trace=False):
    from concourse import bass_utils

    nc = _get_nc()
    in_maps = _prep_in_maps(x, w_qkv, w_proj, b_proj)
    br = bass_utils.run_bass_kernel_spmd(
        nc, in_maps, core_ids=list(range(NCORES)), trace=trace
    )
    y = np.empty((B, N, C), np.float32)
    for core in range(NCORES):
        b, half = core // 2, core % 2
        y[b, half * IQ : (half + 1) * IQ, :] = br.results[core]["out"]
    return y, br


def kernel(x, w_qkv, w_proj, b_proj):
    y, _ = run(x, w_qkv, w_proj, b_proj, trace=False)
    return y
